# revision 41
# baseline (speedup 1.0000x reference)
"""Trainium2 Bass kernel for causal multi-head attention.

Problem: x[1,4096,1024] -> MHA(16 heads, head_dim 64, causal) -> out[1,4096,1024]
  q,k,v = x @ W_{q,k,v}; scores = q k^T / 8 (causal); out = softmax(scores) v @ W_o + b_o

Sharding: tensor-parallel over heads, 2 heads (128 feature dims) per core.

Dataflow (fused single sweep over 512-row query chunks):
  per chunk c: project QKV for chunk c (K appended to persistent K^T, V
  transposed into per-tile V|ones "augmented" blocks), then stream causal
  attention rows qs=c over key tiles kt=0..4c+3 with the transposed-score
  layout (S^T = K Q^T, exp on ACT, per-q softmax sums via the ones column
  in the PV matmul), then the out-projection for chunk c.

  Out-projection: raw ctx^T/sums are evicted, PE-transposed to put q on
  partitions, normalized there by the DVE-reciprocal of the sums column,
  transposed back, and fed as a single 128-contraction matmul per 512-wide
  output chunk: out_c = ctxn_c @ W_o[slice_c], summed over cores on host.

  QKV(c+1) and outproj(c-1) are emitted as "filler units" woven between the
  kt steps of attention(c) (paced uniformly), so the in-order PE queue always
  has independent work while ACT runs exp. Diagonal key tiles skip the
  fully-masked leading query columns in S/exp/mask/PV.

kernel(**inputs) takes the FULL unsharded inputs and returns the FULL output.
"""

import sys

import numpy as np

for _p in ("/opt/trn_rl_repo", "/root/.axon_site/_ro/trn_rl_repo"):
    if _p not in sys.path:
        try:
            import concourse  # noqa: F401

            break
        except ImportError:
            sys.path.insert(0, _p)

N_CORES = 8
SEQ = 4096
D = 1024
DC = 128  # per-core slice of the head dim (2 heads x 64)
HD = 64


def build_bass(n=SEQ, d=D):
    """Trace the per-core SPMD Bass program. n = sequence length."""
    import concourse.bacc as bacc
    import concourse.mybir as mybir
    import concourse.tile as tile

    fp32 = mybir.dt.float32
    bf16 = mybir.dt.bfloat16
    Exp = mybir.ActivationFunctionType.Exp
    Copy = mybir.ActivationFunctionType.Copy

    assert n % 512 == 0 and d % 128 == 0
    NT = n // 128  # 128-row seq tiles
    NCH = n // 512  # 512-col seq chunks
    DIT = d // 128  # input-dim 128-tiles
    # Fold an extra 1/16 into Q so S = s/16; exp runs with scale=16.
    SCALE16 = 1.0 / (float(np.sqrt(HD)) * 16.0)

    nc = bacc.Bacc("TRN2", target_bir_lowering=False)

    xT_d = nc.dram_tensor("xT", (d, n), bf16, kind="ExternalInput")
    wq_d = nc.dram_tensor("wq", (d, DC), bf16, kind="ExternalInput")
    wk_d = nc.dram_tensor("wk", (d, DC), bf16, kind="ExternalInput")
    wv_d = nc.dram_tensor("wv", (d, DC), bf16, kind="ExternalInput")
    wo_d = nc.dram_tensor("wo", (DC, d), bf16, kind="ExternalInput")
    # identity + causal staircase masks precomputed on host: avoids a serial
    # GPSIMD memset/affine_select preamble stalling the first diagonal tile
    ident_d = nc.dram_tensor("ident", (128, 128), bf16, kind="ExternalInput")
    # masks duplicated per head: [dd][h][ql] so one DVE op masks both heads
    masks_d = nc.dram_tensor("masks", (128, 4 * 2 * 512), bf16, kind="ExternalInput")
    out_d = nc.dram_tensor("out", (n, d), fp32, kind="ExternalOutput")

    with tile.TileContext(nc) as tc:
        with (
            tc.tile_pool(name="const", bufs=1) as const_pool,
            tc.tile_pool(name="weights", bufs=1) as w_pool,
            tc.tile_pool(name="big", bufs=1) as big_pool,
            tc.tile_pool(name="xin", bufs=2) as xin_pool,
            tc.tile_pool(name="qt", bufs=2) as qt_pool,
            tc.tile_pool(name="vt", bufs=2) as vt_pool,
            tc.tile_pool(name="pw", bufs=4) as p_pool,
            tc.tile_pool(name="ctxsb", bufs=2) as cs_pool,
            tc.tile_pool(name="norm", bufs=4) as nm_pool,
            tc.tile_pool(name="outsb", bufs=4) as out_pool,
            tc.tile_pool(name="psum", bufs=1, space="PSUM") as ps,
        ):
            # ---- constants (DMA'd from host) ----
            ident = const_pool.tile([128, 128], bf16)
            nc.sync.dma_start(ident[:], ident_d[:])
            # Diagonal causal masks: mask[dd][h][kl, ql] = 1 if ql >= kl + 128*dd
            masks = const_pool.tile([128, 4, 2, 512], bf16)
            nc.sync.dma_start(
                masks[:], masks_d[:].rearrange("p (a h b) -> p a h b", a=4, h=2)
            )

            # ---- weights ----
            wq_sb = w_pool.tile([128, DIT, DC], bf16)
            wk_sb = w_pool.tile([128, DIT, DC], bf16)
            wv_sb = w_pool.tile([128, DIT, DC], bf16)
            nc.sync.dma_start(wq_sb[:], wq_d[:].rearrange("(t p) c -> p t c", p=128))
            nc.sync.dma_start(wk_sb[:], wk_d[:].rearrange("(t p) c -> p t c", p=128))
            nc.sync.dma_start(wv_sb[:], wv_d[:].rearrange("(t p) c -> p t c", p=128))
            wo_sb = w_pool.tile([DC, d], bf16)
            nc.sync.dma_start(wo_sb[:], wo_d[:])

            # ---- persistent activations ----
            kt_sb = big_pool.tile([DC, n], bf16)  # K^T (head dims on partitions)
            # V natural per 128-tile, augmented with ones columns at 64 (h0)
            # and 129 (h1); sums ride along the PV matmul.
            v_aug = big_pool.tile([128, NT, 130], bf16)
            onescol = const_pool.tile([128, NT], fp32)
            nc.gpsimd.memset(onescol[:], 1.0)
            nc.vector.tensor_copy(v_aug[:, :, HD], onescol[:])
            nc.vector.tensor_copy(v_aug[:, :, 2 * HD + 1], onescol[:])

            def start_xch(c, split=False):
                """Prefetch x chunk c (issued ~one chunk ahead of use).
                split=True issues one DMA per dit tile so the first projection
                matmul can start after ~1/8 of the transfer (cold start)."""
                c0, c1 = c * 512, c * 512 + 512
                src = xT_d[:, c0:c1].rearrange("(t p) c -> p t c", p=128)
                xch = xin_pool.tile([128, DIT, 512], bf16, tag="xch", bufs=2)
                if split:
                    for t in range(DIT):
                        nc.sync.dma_start(xch[:, t, :], src[:, t, :])
                else:
                    nc.sync.dma_start(xch[:], src)
                return xch

            def gen_qkv(c, qt_c, xch):
                """Project Q/K/V for 512-row chunk c, as weavable units."""
                c0, c1 = c * 512, c * 512 + 512
                for w_sb, kind in ((wq_sb, "q"), (wk_sb, "k"), (wv_sb, "v")):
                    pp = ps.tile(
                        [DC, 512], fp32, tag="flex", bufs=2,
                        padded_shape=[DC, 512], name=f"p{kind}{c}",
                    )
                    for dit in range(DIT):
                        nc.tensor.matmul(
                            pp[:], w_sb[:, dit, :], xch[:, dit, :],
                            start=(dit == 0), stop=(dit == DIT - 1),
                        )
                    # NOTE: a flex-tag tile's alloc..last-emitted-consumer must
                    # stay within one unit (no yield mid-lifecycle), or a later
                    # flex alloc can WAR-stall the in-order PE queue on ops
                    # emitted after it -> deadlock.
                    if kind == "q":
                        nc.scalar.activation(qt_c[:], pp[:], Copy, scale=SCALE16)
                    elif kind == "k":
                        nc.scalar.activation(kt_sb[:, c0:c1], pp[:], Copy)
                    else:
                        vt_t = vt_pool.tile([DC, 512], bf16, tag="vt", bufs=2)
                        nc.vector.tensor_copy(vt_t[:], pp[:])
                        yield
                        for j in range(4):
                            ti = c * 4 + j
                            tpv = ps.tile(
                                [128, 128], bf16, tag="flex", bufs=2,
                                padded_shape=[128, 1024], name=f"tpv{ti}",
                            )
                            nc.tensor.transpose(
                                tpv[:], vt_t[:, j * 128 : (j + 1) * 128], ident[:]
                            )
                            nc.vector.tensor_copy(v_aug[:, ti, 0:HD], tpv[:, 0:HD])
                            nc.vector.tensor_copy(
                                v_aug[:, ti, HD + 1 : 2 * HD + 1],
                                tpv[:, HD : 2 * HD],
                            )
                            yield
                    yield

            def emit_attention(c, qt_c, filler, n_units):
                """Causal attention for chunk c's query rows; between kt steps,
                drain `filler` (a list of generators: qkv(c+1), outproj(c-1))
                so the in-order PE queue always has independent work while ACT
                runs exp."""
                nkt = 4 * (c + 1)
                ctxm = ps.tile([HD + 1, 2, 512], fp32, tag="ctx", bufs=1)

                def fill_one():
                    while filler:
                        try:
                            next(filler[0])
                            return True
                        except StopIteration:
                            filler.pop(0)
                    return False

                # Pace the filler units uniformly across the nkt steps (late
                # chunks have more steps than units — don't burn all filler
                # early and leave the tail steps PE-starved).
                emitted = 0
                for kt in range(nkt):
                    # Filler emitted BEFORE this step's S so the
                    # S -> exp -> mask -> PV chain stays tight (pm buffers
                    # recycle promptly; the exp stream never waits on a deep
                    # filler queue).
                    want = (kt + 1) * n_units // nkt
                    while emitted < want and fill_one():
                        emitted += 1
                    kc = slice(kt * 128, kt * 128 + 128)
                    dd = kt - 4 * c
                    # Diagonal tiles with dd>=1: query columns [0, 128*dd)
                    # are fully above the causal line -> skip them in S, exp,
                    # mask and PV (their ctx contribution is exactly zero).
                    q0 = 128 * dd if dd >= 1 else 0
                    qs = slice(q0, 512)
                    sm = ps.tile([128, 2, 512], fp32, tag="s", bufs=2)
                    nc.tensor.matmul(
                        sm[:, 0, qs], kt_sb[0:HD, kc], qt_c[0:HD, qs],
                        start=True, stop=True, tile_position=(0, 0),
                    )
                    nc.tensor.matmul(
                        sm[:, 1, qs], kt_sb[HD:DC, kc], qt_c[HD:DC, qs],
                        start=True, stop=True, tile_position=(64, 0),
                    )
                    pm = p_pool.tile([128, 2, 512], bf16, tag="p", bufs=8)
                    nc.scalar.activation(pm[:, :, qs], sm[:, :, qs], Exp, scale=16.0)
                    if dd >= 0:
                        nc.vector.tensor_mul(
                            pm[:, :, qs], pm[:, :, qs], masks[:, dd, :, qs]
                        )
                    nc.tensor.matmul(
                        ctxm[:, 0, qs], v_aug[:, kt, 0 : HD + 1], pm[:, 0, qs],
                        start=(kt == 0), stop=(kt == nkt - 1),
                    )
                    nc.tensor.matmul(
                        ctxm[:, 1, qs],
                        v_aug[:, kt, HD + 1 : 2 * HD + 2], pm[:, 1, qs],
                        start=(kt == 0), stop=(kt == nkt - 1),
                    )
                # Evict raw ctx^T+sums NOW: ctxm is single-buffered, and the
                # next chunk's first PV would otherwise stall on this copy.
                # Two half-copies so the next chunk's h0/h1 PVs unblock
                # incrementally.
                ctxs = cs_pool.tile([HD + 1, 2, 512], bf16, tag="cs", bufs=2)
                nc.vector.tensor_copy(ctxs[:, 0, :], ctxm[:, 0, :])
                nc.vector.tensor_copy(ctxs[:, 1, :], ctxm[:, 1, :])
                while filler:
                    try:
                        next(filler[0])
                    except StopIteration:
                        filler.pop(0)
                return ctxs

            def gen_outproj(c, ctxs):
                """Normalize ctx (q on partitions) and project: out_c = ctxn Wo.

                For the last chunk there is no following attention to hide
                this chain under, and ACT is idle after the final exp — run
                the copies/normalize there instead of DVE to shorten the tail.
                """
                on_act = c == NCH - 1
                for j in range(4):
                    jj = c * 4 + j
                    gsl = slice(jj * 128, jj * 128 + 128)
                    tp = ps.tile(
                        [128, 2, HD + 1], bf16, tag="flex", bufs=2,
                        padded_shape=[128, 2, 512], name=f"tp{jj}",
                    )
                    for h in range(2):
                        nc.tensor.transpose(
                            tp[:, h, :],
                            ctxs[:, h, j * 128 : (j + 1) * 128],
                            ident[0 : HD + 1, 0 : HD + 1],
                        )
                    # Per-q reciprocal of the softmax sums (column 64 of each
                    # head's transposed block), q on partitions.
                    rcp = nm_pool.tile([128, 2], fp32, tag="rcp", bufs=4)
                    nc.vector.reciprocal(rcp[:], tp[:, :, HD])
                    yield
                    # Normalized ctx, heads packed on the free dim: [q, 128]
                    ctxn = nm_pool.tile([128, DC], bf16, tag="cn", bufs=4)
                    if on_act:
                        nc.scalar.activation(
                            ctxn[:, 0:HD], tp[:, 0, 0:HD], Copy, scale=rcp[:, 0:1]
                        )
                        nc.scalar.activation(
                            ctxn[:, HD:DC], tp[:, 1, 0:HD], Copy, scale=rcp[:, 1:2]
                        )
                    else:
                        nc.vector.tensor_scalar_mul(
                            ctxn[:, 0:HD], tp[:, 0, 0:HD], rcp[:, 0:1]
                        )
                        nc.vector.tensor_scalar_mul(
                            ctxn[:, HD:DC], tp[:, 1, 0:HD], rcp[:, 1:2]
                        )
                    # Back to contraction layout [128 ctx-dims, 128 q]
                    ctxnT = ps.tile(
                        [128, 128], bf16, tag="flex", bufs=2,
                        padded_shape=[128, 1024], name=f"cT{jj}",
                    )
                    nc.tensor.transpose(ctxnT[:], ctxn[:], ident[:])
                    ctxf = nm_pool.tile([DC, 128], bf16, tag="cf", bufs=4)
                    if on_act:
                        nc.scalar.copy(ctxf[:], ctxnT[:])
                    else:
                        nc.vector.tensor_copy(ctxf[:], ctxnT[:])
                    yield
                    for h2 in range(d // 512):
                        osl = slice(h2 * 512, (h2 + 1) * 512)
                        op = ps.tile(
                            [128, 512], fp32, tag="flex", bufs=2,
                            padded_shape=[128, 512], name=f"op{jj}_{h2}",
                        )
                        nc.tensor.matmul(
                            op[:], ctxf[:], wo_sb[:, osl], start=True, stop=True
                        )
                        o_sb = out_pool.tile([128, 512], fp32, tag="o", bufs=4)
                        if on_act and h2 == 0:
                            nc.scalar.copy(o_sb[:], op[:])
                        else:
                            nc.vector.tensor_copy(o_sb[:], op[:])
                        nc.sync.dma_start(out_d[gsl, osl], o_sb[:])
                        yield

            # ---- fused sweep ----
            # qt double-buffer tiles created up front so qkv(c+1) can fill
            # attention(c) while attention reads qt(c).
            qts = [
                qt_pool.tile([DC, 512], bf16, tag="qt", bufs=2, name=f"qt{i}")
                for i in range(2)
            ]
            N_QKV_UNITS, N_OP_UNITS = 7, 16
            xch = start_xch(0, split=True)
            xch_next = start_xch(1)
            for _ in gen_qkv(0, qts[0], xch):
                pass
            pending = None  # outproj generator for the previous chunk
            for c in range(NCH):
                filler = []
                n_units = 0
                if c + 1 < NCH:
                    filler.append(gen_qkv(c + 1, qts[(c + 1) % 2], xch_next))
                    n_units += N_QKV_UNITS
                if c + 2 < NCH:
                    xch_next = start_xch(c + 2)
                if pending is not None:
                    filler.append(pending)
                    n_units += N_OP_UNITS
                ctxs = emit_attention(c, qts[c % 2], filler, n_units)
                pending = gen_outproj(c, ctxs)
            for _ in pending:
                pass

    nc.compile()
    return nc


_NC_CACHE = {}


def _get_nc(n=SEQ):
    if n not in _NC_CACHE:
        _NC_CACHE[n] = build_bass(n)
    return _NC_CACHE[n]


def make_in_maps(x, W_q, W_k, W_v, W_o):
    import ml_dtypes

    bf16 = ml_dtypes.bfloat16
    n = x.shape[-2]
    xT = np.ascontiguousarray(
        np.asarray(x, dtype=np.float32).reshape(n, D).T
    ).astype(bf16)
    ident = np.eye(128, dtype=bf16)
    # masks[kl, (dd*2 + h)*512 + ql] = 1 if ql >= kl + 128*dd else 0
    # (duplicated across h so one DVE op masks both heads' pm halves)
    kl = np.arange(128)[:, None]
    ql = np.arange(512)[None, :]
    masks = np.concatenate(
        [(ql >= kl + 128 * dd) for dd in range(4) for _ in range(2)], axis=1
    ).astype(bf16)
    masks = np.ascontiguousarray(masks)
    in_maps = []
    for c in range(N_CORES):
        s = slice(c * DC, (c + 1) * DC)
        in_maps.append(
            {
                "xT": xT,
                "ident": ident,
                "masks": masks,
                "wq": np.ascontiguousarray(np.asarray(W_q, np.float32)[:, s]).astype(bf16),
                "wk": np.ascontiguousarray(np.asarray(W_k, np.float32)[:, s]).astype(bf16),
                "wv": np.ascontiguousarray(np.asarray(W_v, np.float32)[:, s]).astype(bf16),
                "wo": np.ascontiguousarray(np.asarray(W_o, np.float32)[s, :]).astype(bf16),
            }
        )
    return in_maps


def kernel(x, W_q, W_k, W_v, W_o, b_o):
    from concourse import bass_utils

    x = np.asarray(x)
    b, n, _ = x.shape
    assert b == 1 and n == SEQ

    nc = _get_nc(n)
    in_maps = make_in_maps(x, W_q, W_k, W_v, W_o)
    res = bass_utils.run_bass_kernel_spmd(nc, in_maps, list(range(N_CORES)))
    acc = np.zeros((n, D), dtype=np.float64)
    for r in res.results:
        acc += r["out"].astype(np.float64)
    acc += np.asarray(b_o, np.float64)[None, :]
    return acc.astype(np.float32).reshape(1, n, D)
